# revision 1
# baseline (speedup 1.0000x reference)
"""Chamfer distance (nn_ChamferDistance) Trainium2 Bass kernel.

Computes, for xyz1/xyz2 of shape (4, 8192, 3) fp32:
    dist[n, m] = |p_n|^2 + |q_m|^2 - 2 p_n.q_m   (per batch)
    dist1 = min over m, dist2 = min over n
Returns (dist1, dist2), each (4, 8192) fp32 — same as the reference.

Strategy:
  - The pairwise-distance matrix is produced directly by the TensorEngine via
    an augmented inner product: u_a . v_b = sq(P)[a] + sq(Q)[b] - 2 P_a.Q_b.
    All factors are split into 3 bf16 planes (hi/lo/lolo) so every product the
    PE forms is exact in fp32; dropped cross terms are ~2^-26 relative.  K=24
    contraction rows, bf16: a [128x512] distance tile costs ~512 PE cycles.
  - Sharding: 8 cores = 4 batches x 2 halves.  Each core runs TWO layouts:
      A: partitions = its half of N, free = all M  -> dist1 rows (min over free)
      B: partitions = its half of M, free = all N  -> dist2 rows (min over free)
    so both outputs are pure free-axis min-reductions; no partition reduce and
    no cross-core combine is needed.
  - Per 128-row tile, matmuls fill PSUM groups of [128, 1024] (2 banks,
    4-deep pool for overlap).  The ScalarEngine copies each group to SBUF
    (freeing the PSUM bank and taking the PSUM-port load off the DVE), then a
    single VectorEngine tensor_scalar with a min-accumulator produces the
    group's per-row min; a tiny reduce folds the group mins per tile.
"""

import numpy as np
import ml_dtypes

import concourse.bacc as bacc
import concourse.tile as tile
import concourse.mybir as mybir
from concourse import bass_utils

B = 4
N = 8192
M = 8192
NCORES = 8
NSH = N // 2          # rows per core per layout
K = 24                # augmented contraction rows

BF16 = mybir.dt.bfloat16
F32 = mybir.dt.float32
MIN = mybir.AluOpType.min
ADD = mybir.AluOpType.add
X = mybir.AxisListType.X
BIG = 1.0e30


def _emit_layout(tc, pools, lhs_sb, rhs_sb, dst, nt, m, gf):
    """One layout: dst[:, i] = min over free of (lhsT[:, i-tile].T @ rhs)."""
    nc = tc.nc
    ng = m // gf
    nj = gf // 512
    psum_pool, stage_pool, rowm_pool = pools
    for i in range(nt):
        # ACT stages each PSUM group into one [128, m] SBUF row (freeing the
        # PSUM banks early and taking the PSUM-read load off the VectorEngine);
        # a single DVE tensor_scalar min-accumulator then reduces the whole
        # row straight into dst[:, i].
        st = stage_pool.tile([128, m], F32, tag="st")
        for g in range(ng):
            ps = psum_pool.tile([128, gf], F32, tag="ps")
            for jj in range(nj):
                nc.tensor.matmul(
                    ps[:, jj * 512:(jj + 1) * 512],
                    lhs_sb[:, i * 128:(i + 1) * 128],
                    rhs_sb[:, g * gf + jj * 512: g * gf + (jj + 1) * 512],
                    start=True,
                    stop=True,
                )
            nc.scalar.copy(st[:, g * gf:(g + 1) * gf], ps[:])
        scr = stage_pool.tile([128, m], F32, tag="scr")
        nc.vector.tensor_scalar(
            scr[:], st[:], 0.0, None, op0=ADD, op1=MIN,
            accum_out=dst[:, i:i + 1])


def build_body(tc, lhsT_a, rhs_a, lhsT_b, rhs_b, d1t, d2t, nt, m, gf, repeat=1):
    """Emit the kernel body into TileContext `tc`.

    lhsT_a: [K, nt*128] bf16 AP  (augmented rows of this core's N-half)
    rhs_a:  [K, m]      bf16 AP  (augmented rows of all of xyz2)
    lhsT_b: [K, nt*128] bf16 AP  (augmented rows of this core's M-half)
    rhs_b:  [K, m]      bf16 AP  (augmented rows of all of xyz1)
    d1t, d2t: [128, nt] f32 APs out (row r of tile i -> point i*128 + r)
    """
    nc = tc.nc
    with (
        tc.tile_pool(name="inp", bufs=1) as inp_pool,
        tc.tile_pool(name="acc", bufs=1) as acc_pool,
        tc.tile_pool(name="rowm", bufs=8) as rowm_pool,
        tc.tile_pool(name="stage", bufs=2) as stage_pool,
        tc.tile_pool(name="psum", bufs=8 // (gf // 512), space="PSUM") as psum_pool,
    ):
        las = inp_pool.tile([K, nt * 128], BF16, tag="la")
        nc.sync.dma_start(las[:], lhsT_a)
        ras = inp_pool.tile([K, m], BF16, tag="ra")
        nc.sync.dma_start(ras[:], rhs_a)
        lbs = inp_pool.tile([K, nt * 128], BF16, tag="lb")
        nc.sync.dma_start(lbs[:], lhsT_b)
        rbs = inp_pool.tile([K, m], BF16, tag="rb")
        nc.sync.dma_start(rbs[:], rhs_b)

        d1 = acc_pool.tile([128, nt], F32, tag="d1")
        d2 = acc_pool.tile([128, nt], F32, tag="d2")

        pools = (psum_pool, stage_pool, rowm_pool)
        for _ in range(repeat):
            _emit_layout(tc, pools, las, ras, d1, nt, m, gf)
            _emit_layout(tc, pools, lbs, rbs, d2, nt, m, gf)

        nc.sync.dma_start(d1t, d1[:])
        nc.sync.dma_start(d2t, d2[:])


def build_kernel(nc, nt=NSH // 128, m=M, gf=1024, repeat=1):
    lhsT_a = nc.dram_tensor("lhsT_a", [K, nt * 128], BF16, kind="ExternalInput")
    rhs_a = nc.dram_tensor("rhs_a", [K, m], BF16, kind="ExternalInput")
    lhsT_b = nc.dram_tensor("lhsT_b", [K, nt * 128], BF16, kind="ExternalInput")
    rhs_b = nc.dram_tensor("rhs_b", [K, m], BF16, kind="ExternalInput")
    d1t = nc.dram_tensor("d1t", [128, nt], F32, kind="ExternalOutput")
    d2t = nc.dram_tensor("d2t", [128, nt], F32, kind="ExternalOutput")
    with tile.TileContext(nc) as tc:
        build_body(tc, lhsT_a.ap(), rhs_a.ap(), lhsT_b.ap(), rhs_b.ap(),
                   d1t.ap(), d2t.ap(), nt, m, gf, repeat)
    return nc


def _split3(v):
    """v (fp32) -> three bf16 planes (as fp32) with v ~= h + l + ll."""
    bf = ml_dtypes.bfloat16
    h = v.astype(bf).astype(np.float32)
    l = (v - h).astype(bf).astype(np.float32)
    ll = (v - h - l).astype(bf).astype(np.float32)
    return h, l, ll


def _build_aug(x1, x2):
    """x1 [n,3], x2 [m,3] fp32 -> (L [24,n] bf16, R [24,m] bf16) with
    (L.T @ R)[a,b] ~= |x1_a|^2 + |x2_b|^2 - 2 x1_a.x2_b."""
    n = x1.shape[0]
    m = x2.shape[0]
    sq1 = (x1 * x1).sum(-1)
    sq2 = (x2 * x2).sum(-1)
    a = -2.0 * x1
    y = x2
    s1h, s1l, s1ll = _split3(sq1)
    s2h, s2l, s2ll = _split3(sq2)
    ah, al, all_ = _split3(a)
    yh, yl, yll = _split3(y)
    ones_n = np.ones(n, np.float32)
    ones_m = np.ones(m, np.float32)
    Ls = [s1h, s1l, s1ll, ones_n, ones_n, ones_n]
    Rs = [ones_m, ones_m, ones_m, s2h, s2l, s2ll]
    for c in range(3):
        for (L, R) in ((ah, yh), (ah, yl), (ah, yll), (al, yh), (al, yl), (all_, yh)):
            Ls.append(L[:, c])
            Rs.append(R[:, c])
    bf = ml_dtypes.bfloat16
    Lm = np.ascontiguousarray(np.stack(Ls)).astype(bf)
    Rm = np.ascontiguousarray(np.stack(Rs)).astype(bf)
    return Lm, Rm


def _make_in_maps(xyz1, xyz2):
    in_maps = []
    for c in range(NCORES):
        b, h = divmod(c, 2)
        La, Ra = _build_aug(xyz1[b, h * NSH:(h + 1) * NSH], xyz2[b])
        Lb, Rb = _build_aug(xyz2[b, h * NSH:(h + 1) * NSH], xyz1[b])
        in_maps.append({"lhsT_a": La, "rhs_a": Ra, "lhsT_b": Lb, "rhs_b": Rb})
    return in_maps


_CACHE = {}


def _get_compiled(repeat=1):
    key = ("nc", repeat)
    if key not in _CACHE:
        nc = bacc.Bacc("TRN2", target_bir_lowering=False, debug=False,
                       num_devices=NCORES)
        build_kernel(nc, repeat=repeat)
        nc.compile()
        _CACHE[key] = nc
    return _CACHE[key]


def _gather(results):
    d1 = np.empty((B, N), np.float32)
    d2 = np.empty((B, M), np.float32)
    for c in range(NCORES):
        b, h = divmod(c, 2)
        d1[b, h * NSH:(h + 1) * NSH] = results[c]["d1t"].T.reshape(-1)
        d2[b, h * NSH:(h + 1) * NSH] = results[c]["d2t"].T.reshape(-1)
    return d1, d2


def kernel(xyz1, xyz2):
    xyz1 = np.asarray(xyz1, dtype=np.float32)
    xyz2 = np.asarray(xyz2, dtype=np.float32)
    in_maps = _make_in_maps(xyz1, xyz2)
    nc = _get_compiled()
    res = bass_utils.run_bass_kernel_spmd(nc, in_maps, core_ids=list(range(NCORES)))
    return _gather(res.results)



# revision 5
# speedup vs baseline: 1622.8885x; 1622.8885x over previous
"""Chamfer distance (nn_ChamferDistance) Trainium2 Bass kernel.

Computes, for xyz1/xyz2 of shape (4, 8192, 3) fp32:
    dist[n, m] = |p_n|^2 + |q_m|^2 - 2 p_n.q_m   (per batch)
    dist1 = min over m, dist2 = min over n
Returns (dist1, dist2), each (4, 8192) fp32 — same as the reference.

Strategy (single-pass, negated):
  - The pairwise-distance matrix is produced directly by the TensorEngine via
    an augmented inner product: u_a . v_b = sq(P)[a] + sq(Q)[b] - 2 P_a.Q_b.
    All factors are split into 3 bf16 planes (hi/lo/lolo) so every product the
    PE forms is exact in fp32; dropped cross terms are ~2^-26 relative.  The
    L-side planes are negated on the host so the device computes -dist and
    every reduction becomes a MAX (required by gpsimd partition_all_reduce,
    which has no min).  Host negates the outputs back.
  - Sharding: 8 cores = 4 batches x 2 halves of N.  Each core computes its
    4096 x 8192 block of -dist ONCE (the baseline computed it twice):
      * dist1 rows come from a per-tile free-axis max,
      * dist2 comes from an elementwise column-accumulator max across the 32
        row-tiles, partition-reduced at the end; the two halves of a batch
        are combined on the host (elementwise max of two 8192-vectors).
  - Per 128-row tile, 16 matmuls fill 4 PSUM groups of [128, 2048].  The
    ScalarEngine drains each group to SBUF with an fp32->fp16 downcast
    (fp16 keeps min errors ~2^-11; tolerance is 2e-2).  The VectorEngine then
    consumes each staged tile with 2x-mode tensor_tensor ops only - measured:
    TT fp16 runs at 2 elem/cyc while every reduce-shaped op (tensor_reduce,
    Max8, Pool, tensor_scalar+accum) runs at 1 elem/cyc and
    tensor_tensor_reduce crashes the exec unit:
      * row-max: TT-max fold tree 8192->4096->2048->1024->512 plus one
        1x tensor_reduce on the last 512 (~4.8us vs 8.7us flat reduce).
      * column accumulator: one TT max (4.3us).
  - Tail: gpsimd partition_all_reduce(max) collapses the column accumulator
    across partitions; row 0 is DMA'd out per core.
"""

import numpy as np
import ml_dtypes

import concourse.bacc as bacc
import concourse.tile as tile
import concourse.mybir as mybir
import concourse.bass_isa as bass_isa
from concourse import bass_utils

B = 4
N = 8192
M = 8192
NCORES = 8
NSH = N // 2          # rows per core
NT = NSH // 128       # 32 row tiles per core
K = 24                # augmented contraction rows
GF = 2048             # PSUM drain group size (4 banks; 2 groups in flight)

BF16 = mybir.dt.bfloat16
F16 = mybir.dt.float16
F32 = mybir.dt.float32
MAX = mybir.AluOpType.max
X = mybir.AxisListType.X
NEG_BIG = -3.0e38


def build_body(tc, lhsT, rhs, d1t, d2t, repeat=1):
    """Emit the kernel body into TileContext `tc`.

    lhsT: [K, NT*128] bf16 AP  (negated augmented rows of this core's N-half)
    rhs:  [K, M]      bf16 AP  (augmented rows of all of xyz2[b])
    d1t:  [128, NT] f32 AP out (row r of tile i -> -dist1 of point i*128 + r)
    d2t:  [1, M]   f16 AP out (-dist2 partial max over this core's rows)
    """
    nc = tc.nc
    ng = M // GF
    nj = GF // 512
    with (
        tc.tile_pool(name="inp", bufs=1) as inp_pool,
        tc.tile_pool(name="acc", bufs=1) as acc_pool,
        tc.tile_pool(name="stage", bufs=2) as stage_pool,
        tc.tile_pool(name="scr", bufs=1) as scr_pool,
        tc.tile_pool(name="cacc", bufs=1) as cacc_pool,
        tc.tile_pool(name="psum", bufs=2, space="PSUM") as psum_pool,
    ):
        ls = inp_pool.tile([K, NT * 128], BF16, tag="ls")
        nc.sync.dma_start(ls[:], lhsT)
        rs = inp_pool.tile([K, M], BF16, tag="rs")
        nc.sync.dma_start(rs[:], rhs)

        d1 = acc_pool.tile([128, NT], F32, tag="d1")
        colacc = cacc_pool.tile([128, M], F16, tag="cacc")
        par = cacc_pool.tile([128, M], F16, tag="par")

        for _ in range(repeat):
            for i in range(NT):
                st = stage_pool.tile([128, M], F16, tag="st")
                for g in range(ng):
                    ps = psum_pool.tile([128, GF], F32, tag="ps")
                    for j in range(nj):
                        nc.tensor.matmul(
                            ps[:, j * 512:(j + 1) * 512],
                            ls[:, i * 128:(i + 1) * 128],
                            rs[:, g * GF + j * 512: g * GF + (j + 1) * 512],
                            start=True,
                            stop=True,
                        )
                    nc.scalar.copy(st[:, g * GF:(g + 1) * GF], ps[:])
                scr = scr_pool.tile([128, 7680], F16, tag="scr")
                nc.vector.tensor_tensor(scr[:, :4096], st[:, :4096], st[:, 4096:],
                                        op=MAX)
                nc.vector.tensor_tensor(scr[:, 4096:6144], scr[:, :2048],
                                        scr[:, 2048:4096], op=MAX)
                nc.vector.tensor_tensor(scr[:, 6144:7168], scr[:, 4096:5120],
                                        scr[:, 5120:6144], op=MAX)
                nc.vector.tensor_tensor(scr[:, 7168:7680], scr[:, 6144:6656],
                                        scr[:, 6656:7168], op=MAX)
                nc.vector.tensor_reduce(d1[:, i:i + 1], scr[:, 7168:7680],
                                        axis=X, op=MAX)
                if i == 0:
                    nc.vector.tensor_copy(colacc[:], st[:])
                else:
                    nc.vector.tensor_tensor(colacc[:], st[:], colacc[:], op=MAX)

            nc.gpsimd.partition_all_reduce(par[:], colacc[:], 128,
                                           bass_isa.ReduceOp.max)

        nc.sync.dma_start(d1t, d1[:])
        nc.sync.dma_start(d2t, par[0:1, :])


def build_kernel(nc, repeat=1):
    lhsT = nc.dram_tensor("lhsT", [K, NT * 128], BF16, kind="ExternalInput")
    rhs = nc.dram_tensor("rhs", [K, M], BF16, kind="ExternalInput")
    d1t = nc.dram_tensor("d1t", [128, NT], F32, kind="ExternalOutput")
    d2t = nc.dram_tensor("d2t", [1, M], F16, kind="ExternalOutput")
    with tile.TileContext(nc) as tc:
        build_body(tc, lhsT.ap(), rhs.ap(), d1t.ap(), d2t.ap(), repeat)
    return nc


def _split3(v):
    """v (fp32) -> three bf16 planes (as fp32) with v ~= h + l + ll."""
    bf = ml_dtypes.bfloat16
    h = v.astype(bf).astype(np.float32)
    l = (v - h).astype(bf).astype(np.float32)
    ll = (v - h - l).astype(bf).astype(np.float32)
    return h, l, ll


def _build_aug(x1, x2):
    """x1 [n,3], x2 [m,3] fp32 -> (L [24,n] bf16, R [24,m] bf16) with
    (L.T @ R)[a,b] ~= -(|x1_a|^2 + |x2_b|^2 - 2 x1_a.x2_b)  (negated)."""
    n = x1.shape[0]
    m = x2.shape[0]
    sq1 = (x1 * x1).sum(-1)
    sq2 = (x2 * x2).sum(-1)
    a = -2.0 * x1
    y = x2
    s1h, s1l, s1ll = _split3(sq1)
    s2h, s2l, s2ll = _split3(sq2)
    ah, al, all_ = _split3(a)
    yh, yl, yll = _split3(y)
    ones_n = np.ones(n, np.float32)
    ones_m = np.ones(m, np.float32)
    Ls = [s1h, s1l, s1ll, ones_n, ones_n, ones_n]
    Rs = [ones_m, ones_m, ones_m, s2h, s2l, s2ll]
    for c in range(3):
        for (L, R) in ((ah, yh), (ah, yl), (ah, yll), (al, yh), (al, yl), (all_, yh)):
            Ls.append(L[:, c])
            Rs.append(R[:, c])
    bf = ml_dtypes.bfloat16
    Lm = np.ascontiguousarray(-np.stack(Ls)).astype(bf)   # negated
    Rm = np.ascontiguousarray(np.stack(Rs)).astype(bf)
    return Lm, Rm


def _make_in_maps(xyz1, xyz2):
    in_maps = []
    for c in range(NCORES):
        b, h = divmod(c, 2)
        L, R = _build_aug(xyz1[b, h * NSH:(h + 1) * NSH], xyz2[b])
        in_maps.append({"lhsT": L, "rhs": R})
    return in_maps


_CACHE = {}


def _get_compiled(repeat=1):
    key = ("nc", repeat)
    if key not in _CACHE:
        nc = bacc.Bacc("TRN2", target_bir_lowering=False, debug=False,
                       num_devices=NCORES)
        build_kernel(nc, repeat=repeat)
        nc.compile()
        _CACHE[key] = nc
    return _CACHE[key]


def _gather(results):
    d1 = np.empty((B, N), np.float32)
    d2 = np.empty((B, M), np.float32)
    for b in range(B):
        r0 = results[2 * b]
        r1 = results[2 * b + 1]
        d1[b, :NSH] = -r0["d1t"].T.reshape(-1)
        d1[b, NSH:] = -r1["d1t"].T.reshape(-1)
        m0 = r0["d2t"][0].astype(np.float32)
        m1 = r1["d2t"][0].astype(np.float32)
        d2[b] = -np.maximum(m0, m1)
    return d1, d2


def kernel(xyz1, xyz2):
    xyz1 = np.asarray(xyz1, dtype=np.float32)
    xyz2 = np.asarray(xyz2, dtype=np.float32)
    in_maps = _make_in_maps(xyz1, xyz2)
    nc = _get_compiled()
    res = bass_utils.run_bass_kernel_spmd(nc, in_maps, core_ids=list(range(NCORES)))
    return _gather(res.results)


# revision 10
# speedup vs baseline: 1697.8659x; 1.0462x over previous
"""Chamfer distance (nn_ChamferDistance) Trainium2 Bass kernel.

Computes, for xyz1/xyz2 of shape (4, 8192, 3) fp32:
    dist[n, m] = |p_n|^2 + |q_m|^2 - 2 p_n.q_m   (per batch)
    dist1 = min over m, dist2 = min over n
Returns (dist1, dist2), each (4, 8192) fp32 — same as the reference.

Strategy (single-pass, negated):
  - The pairwise-distance matrix is produced directly by the TensorEngine via
    an augmented inner product: u_a . v_b = sq(P)[a] + sq(Q)[b] - 2 P_a.Q_b.
    All factors are split into 3 bf16 planes (hi/lo/lolo) so every product the
    PE forms is exact in fp32; dropped cross terms are ~2^-26 relative.  The
    L-side planes are negated on the host so the device computes -dist and
    every reduction becomes a MAX (required by gpsimd partition_all_reduce,
    which has no min).  Host negates the outputs back.
  - Sharding: 8 cores = 4 batches x 2 halves of N.  Each core computes its
    4096 x 8192 block of -dist ONCE (the baseline computed it twice):
      * dist1 rows come from a per-tile free-axis max,
      * dist2 comes from an elementwise column-accumulator max across the 32
        row-tiles, partition-reduced at the end; the two halves of a batch
        are combined on the host (elementwise max of two 8192-vectors).
  - Per 128-row tile, 16 matmuls fill 4 PSUM groups of [128, 2048].  The
    ScalarEngine drains each group to SBUF with an fp32->fp16 downcast
    (fp16 keeps min errors ~2^-11; tolerance is 2e-2).  The VectorEngine then
    consumes each staged tile with 2x-mode tensor_tensor ops only - measured:
    TT fp16 runs at 2 elem/cyc while every reduce-shaped op (tensor_reduce,
    Max8, Pool, tensor_scalar+accum) runs at 1 elem/cyc and
    tensor_tensor_reduce crashes the exec unit:
      * row-max: TT-max fold tree 8192->4096->2048->1024->512 plus one
        1x tensor_reduce on the last 512 (~4.8us vs 8.7us flat reduce).
      * column accumulator: one TT max (4.3us).
  - Tail: gpsimd partition_all_reduce(max) collapses the column accumulator
    across partitions; row 0 is DMA'd out per core.
"""

import numpy as np
import ml_dtypes

import concourse.bacc as bacc
import concourse.tile as tile
import concourse.mybir as mybir
import concourse.bass_isa as bass_isa
from concourse import bass_utils

B = 4
N = 8192
M = 8192
NCORES = 8
NSH = N // 2          # rows per core
NT = NSH // 128       # 32 row tiles per core
K = 24                # augmented contraction rows
GF = 2048             # PSUM drain group size (4 banks; 2 groups in flight)

BF16 = mybir.dt.bfloat16
F16 = mybir.dt.float16
F32 = mybir.dt.float32
MAX = mybir.AluOpType.max
X = mybir.AxisListType.X
NEG_BIG = -3.0e38


def build_body(tc, lhsT, rhs, d1t, d2t, repeat=1):
    """Emit the kernel body into TileContext `tc`.

    lhsT: [K, NT*128] bf16 AP  (negated augmented rows of this core's N-half)
    rhs:  [K, M]      bf16 AP  (augmented rows of all of xyz2[b])
    d1t:  [128, NT] f32 AP out (row r of tile i -> -dist1 of point i*128 + r)
    d2t:  [1, M]   f16 AP out (-dist2 partial max over this core's rows)
    """
    nc = tc.nc
    ng = M // GF
    nj = GF // 512
    with (
        tc.tile_pool(name="inp", bufs=1) as inp_pool,
        tc.tile_pool(name="acc", bufs=1) as acc_pool,
        tc.tile_pool(name="stage", bufs=4) as stage_pool,
        tc.tile_pool(name="scr", bufs=1) as scr_pool,
        tc.tile_pool(name="cacc", bufs=1) as cacc_pool,
        tc.tile_pool(name="psum", bufs=2, space="PSUM") as psum_pool,
    ):
        ls = inp_pool.tile([K, NT * 128], BF16, tag="ls")
        nc.sync.dma_start(ls[:], lhsT)
        rs = inp_pool.tile([K, M], BF16, tag="rs")
        nc.sync.dma_start(rs[:], rhs)

        d1 = acc_pool.tile([128, NT], F32, tag="d1")
        colacc = cacc_pool.tile([128, M], F16, tag="cacc")
        par = cacc_pool.tile([128, M], F16, tag="par")

        QT = 4   # row-tiles per scr group
        for _ in range(repeat):
            for ip in range(NT // QT):
                # QT row-tiles share one scr buffer so tree levels L2..L5 and
                # the final 1x reduce run once per group over [128, QT, n] APs
                # (the 2x-1p mode check only looks at the innermost dim).
                # scr layout per tile: [0:4096] L1, [4096:6144] L2,
                # [6144:7168] L3, [7168:7680] L4, [7680:7936] L5.
                scr = scr_pool.tile([128, QT, 7936], F16, tag="scr")
                for k in range(QT):
                    i = QT * ip + k
                    st = stage_pool.tile([128, M], F16, tag="st")
                    for g in range(ng):
                        ps = psum_pool.tile([128, GF], F32, tag="ps")
                        for j in range(nj):
                            nc.tensor.matmul(
                                ps[:, j * 512:(j + 1) * 512],
                                ls[:, i * 128:(i + 1) * 128],
                                rs[:, g * GF + j * 512: g * GF + (j + 1) * 512],
                                start=True,
                                stop=True,
                            )
                        nc.scalar.copy(st[:, g * GF:(g + 1) * GF], ps[:])
                    nc.vector.tensor_tensor(scr[:, k, :4096], st[:, :4096],
                                            st[:, 4096:], op=MAX)
                    if i == 0:
                        nc.vector.tensor_copy(colacc[:], st[:])
                    else:
                        nc.vector.tensor_tensor(colacc[:], st[:], colacc[:],
                                                op=MAX)
                nc.vector.tensor_tensor(scr[:, :, 4096:6144], scr[:, :, :2048],
                                        scr[:, :, 2048:4096], op=MAX)
                nc.vector.tensor_tensor(scr[:, :, 6144:7168], scr[:, :, 4096:5120],
                                        scr[:, :, 5120:6144], op=MAX)
                nc.vector.tensor_tensor(scr[:, :, 7168:7680], scr[:, :, 6144:6656],
                                        scr[:, :, 6656:7168], op=MAX)
                nc.vector.tensor_tensor(scr[:, :, 7680:7936], scr[:, :, 7168:7424],
                                        scr[:, :, 7424:7680], op=MAX)
                nc.vector.tensor_reduce(d1[:, QT * ip:QT * (ip + 1)],
                                        scr[:, :, 7680:7936], axis=X, op=MAX)

            nc.gpsimd.partition_all_reduce(par[:], colacc[:], 128,
                                           bass_isa.ReduceOp.max)

        nc.sync.dma_start(d1t, d1[:])
        nc.sync.dma_start(d2t, par[0:1, :])


def build_kernel(nc, repeat=1):
    lhsT = nc.dram_tensor("lhsT", [K, NT * 128], BF16, kind="ExternalInput")
    rhs = nc.dram_tensor("rhs", [K, M], BF16, kind="ExternalInput")
    d1t = nc.dram_tensor("d1t", [128, NT], F32, kind="ExternalOutput")
    d2t = nc.dram_tensor("d2t", [1, M], F16, kind="ExternalOutput")
    with tile.TileContext(nc) as tc:
        build_body(tc, lhsT.ap(), rhs.ap(), d1t.ap(), d2t.ap(), repeat)
    return nc


def _split3(v):
    """v (fp32) -> three bf16 planes (as fp32) with v ~= h + l + ll."""
    bf = ml_dtypes.bfloat16
    h = v.astype(bf).astype(np.float32)
    l = (v - h).astype(bf).astype(np.float32)
    ll = (v - h - l).astype(bf).astype(np.float32)
    return h, l, ll


def _build_aug(x1, x2):
    """x1 [n,3], x2 [m,3] fp32 -> (L [24,n] bf16, R [24,m] bf16) with
    (L.T @ R)[a,b] ~= -(|x1_a|^2 + |x2_b|^2 - 2 x1_a.x2_b)  (negated)."""
    n = x1.shape[0]
    m = x2.shape[0]
    sq1 = (x1 * x1).sum(-1)
    sq2 = (x2 * x2).sum(-1)
    a = -2.0 * x1
    y = x2
    s1h, s1l, s1ll = _split3(sq1)
    s2h, s2l, s2ll = _split3(sq2)
    ah, al, all_ = _split3(a)
    yh, yl, yll = _split3(y)
    ones_n = np.ones(n, np.float32)
    ones_m = np.ones(m, np.float32)
    Ls = [s1h, s1l, s1ll, ones_n, ones_n, ones_n]
    Rs = [ones_m, ones_m, ones_m, s2h, s2l, s2ll]
    for c in range(3):
        for (L, R) in ((ah, yh), (ah, yl), (ah, yll), (al, yh), (al, yl), (all_, yh)):
            Ls.append(L[:, c])
            Rs.append(R[:, c])
    bf = ml_dtypes.bfloat16
    Lm = np.ascontiguousarray(-np.stack(Ls)).astype(bf)   # negated
    Rm = np.ascontiguousarray(np.stack(Rs)).astype(bf)
    return Lm, Rm


def _make_in_maps(xyz1, xyz2):
    in_maps = []
    for c in range(NCORES):
        b, h = divmod(c, 2)
        L, R = _build_aug(xyz1[b, h * NSH:(h + 1) * NSH], xyz2[b])
        in_maps.append({"lhsT": L, "rhs": R})
    return in_maps


_CACHE = {}


def _get_compiled(repeat=1):
    key = ("nc", repeat)
    if key not in _CACHE:
        nc = bacc.Bacc("TRN2", target_bir_lowering=False, debug=False,
                       num_devices=NCORES)
        build_kernel(nc, repeat=repeat)
        nc.compile()
        _CACHE[key] = nc
    return _CACHE[key]


def _gather(results):
    d1 = np.empty((B, N), np.float32)
    d2 = np.empty((B, M), np.float32)
    for b in range(B):
        r0 = results[2 * b]
        r1 = results[2 * b + 1]
        d1[b, :NSH] = -r0["d1t"].T.reshape(-1)
        d1[b, NSH:] = -r1["d1t"].T.reshape(-1)
        m0 = r0["d2t"][0].astype(np.float32)
        m1 = r1["d2t"][0].astype(np.float32)
        d2[b] = -np.maximum(m0, m1)
    return d1, d2


def kernel(xyz1, xyz2):
    xyz1 = np.asarray(xyz1, dtype=np.float32)
    xyz2 = np.asarray(xyz2, dtype=np.float32)
    in_maps = _make_in_maps(xyz1, xyz2)
    nc = _get_compiled()
    res = bass_utils.run_bass_kernel_spmd(nc, in_maps, core_ids=list(range(NCORES)))
    return _gather(res.results)
